# revision 103
# baseline (speedup 1.0000x reference)
"""Trainium2 Bass kernel for nn_AttentionSampling (sparse window attention block).

Sharding: 8 cores, data-parallel, 1024 windows (half a batch) per core; windows are
independent so there is no cross-core communication. Activations live in a transposed
[d, tokens] layout (host pre-transposes q/k) so every projection runs with the
weight stationary; raw v ships token-major phase-grouped. Matmul operands are bf16
(k-projection fp8 e4m3); attention weights and LN statistics math stay fp32.

Key structural points:
- v-projection is LINEAR, so the weighted window-sum commutes before it:
  the kernel reduces 4096 raw v tokens -> 1024 on the DVE (stt chain with
  per-window band-masked score weights), transposes the downsample on the
  PE, and projects only 1024 tokens. The (zero here) v-bias folds in as
  u = Wv^-T bv pre-projection when nonzero.
- The q-residual add rides the v-projection PSUM as an identity-matmul
  accumulation step.
- Band-masked scores + strided reduction extract the 4 per-window dot
  products as per-partition scalars.
- LayerNorm runs in the transposed domain with bf16 stats/broadcast
  matmuls (4x cheaper than fp32 on the PE); apply rounds are spread
  across DVE and the Pool engine; LN1(0), which overlaps the attention
  phase, runs entirely off the hot ACT/DVE pair.
- Block mids (PE transposes of the downsample) lag two blocks so the
  serial DVE stt chain can never stall the PE.
- Head: ki-interleaved first loads spread over three DMA issue queues +
  PE p-state warmup matmuls. Tail: per-dt chunked bf16 output DMA.
"""

import sys
import types

# If BASS_TRACE is set in an environment whose antenv package lacks
# axon_hooks, run_bass_kernel_spmd would crash on import; provide a stub
# (a None hook makes bass_utils skip tracing gracefully).
try:
    import antenv.axon_hooks  # noqa: F401
except ImportError:
    _m = types.ModuleType("antenv.axon_hooks")
    _m.get_axon_ntff_profile_hook = lambda: None
    _m.set_axon_ntff_profile_hook = lambda h: None
    sys.modules["antenv.axon_hooks"] = _m
    try:
        import antenv

        antenv.axon_hooks = _m
    except ImportError:
        pass

import contextlib

import numpy as np

import concourse.bass as bass
import concourse.bacc as bacc_mod
import concourse.mybir as mybir
import concourse.tile as tile
from concourse.bass import ts, ds
from concourse.bass_utils import run_bass_kernel_spmd

FP32 = mybir.dt.float32
FP32R = mybir.dt.float32r
AF = mybir.ActivationFunctionType
OP = mybir.AluOpType

MM_DT = mybir.dt.bfloat16  # matmul operands; attention weights/LN stay fp32
FP8 = mybir.dt.float8e4    # k-projection operands (DoubleRow perf mode)

B, SQ, SK, D, F = 4, 2048, 8192, 512, 4
NCORES = 8
WPC = B * SQ // NCORES        # 1024 windows (= tokens) per core
KPC = WPC * F                 # 4096 keys per core
NBLK = WPC // 128             # 8 attention blocks: 128 windows / 512 keys
NSB = WPC // 512              # 2 superblocks of 512 tokens
DT = D // 128                 # 4 d-tiles
EPS = 1e-5

_CACHE = {}


def _ln_stages(nc, P, resid_view, sq_tile, stats_sb, out_cb, n=512, pool_only=False):
    """Transposed LayerNorm over D for an n-token block, as three stages.

    The stages can be emitted at different program points so every PE
    instruction (stats matmuls, broadcast matmuls) enters the in-order PE
    stream only after its slow Pool/ACT input chain has had a full
    attention block of slack. resid_view/sq_tile: [128, DT, n] bf16;
    sq_tile doubles as apply scratch. stats_sb: [1, 2n] bf16 (mean|rstd).
    """
    mean_bf = stats_sb[:, :n]
    rstd_bf = stats_sb[:, n : 2 * n]

    def stage_sq():
        # sq per d-tile, alternating engines: the stats accumulation
        # consumes sq[dt] in order, so early tiles unblock the PE sooner
        sq_engines = [nc.gpsimd] * 4 if pool_only else [nc.vector, nc.gpsimd, nc.vector, nc.gpsimd]
        for dt in range(DT):
            sq_engines[dt].tensor_tensor(
                sq_tile[:, dt, :], resid_view[:, dt, :], resid_view[:, dt, :], op=OP.mult
            )

    def stage_stats():
        st_sum = P["st"].tile([1, 512], FP32, tag="st", name="st_sum")[:, :n]
        for dt in range(DT):
            nc.tensor.matmul(
                st_sum, lhsT=P["ones_col"], rhs=resid_view[:, dt, :],
                start=(dt == 0), stop=(dt == DT - 1),
            )
        nc.scalar.activation(out=mean_bf, in_=st_sum, func=AF.Copy, scale=1.0 / D)

        st_sq = P["st"].tile([1, 512], FP32, tag="st", name="st_sq")[:, :n]
        for dt in range(DT):
            nc.tensor.matmul(
                st_sq, lhsT=P["ones_col"], rhs=sq_tile[:, dt, :],
                start=(dt == 0), stop=(dt == DT - 1),
            )
        e2 = P["small"].tile([1, 512], FP32, tag="e2", name="e2")[:, :n]
        var = P["small"].tile([1, 512], FP32, tag="var", name="var")[:, :n]
        nc.scalar.activation(out=e2, in_=st_sq, func=AF.Copy, scale=1.0 / D)
        nc.vector.tensor_tensor(var, mean_bf, mean_bf, op=OP.mult)
        nc.vector.tensor_tensor(var, e2, var, op=OP.subtract)
        rstd = P["small"].tile([1, 512], FP32, tag="rstd", name="rstd")[:, :n]
        nc.scalar.activation(out=var, in_=var, func=AF.Sqrt, bias=P["eps_t"], scale=1.0)
        nc.vector.reciprocal_approx_fast(out=rstd, in_=var)
        nc.scalar.activation(out=rstd_bf, in_=rstd, func=AF.Copy)

    def stage_apply():
        # broadcast mean/rstd across partitions via PE rank-1, then copy to
        # SBUF bf16 so the apply rounds can run on the Pool engine.
        # bc copies go to DVE when the LN runs in the ACT-hot attention window
        def bc_copy(dst, src):
            if pool_only:
                nc.vector.tensor_scalar(out=dst, in0=src, scalar1=1.0, scalar2=None, op0=OP.mult)
            else:
                nc.scalar.activation(out=dst, in_=src, func=AF.Copy)

        bc = P["bc"].tile([128, 512], FP32, tag="bc", name="bc_mean")[:, :n]
        nc.tensor.matmul(bc, lhsT=P["ones_row"], rhs=mean_bf, start=True, stop=True)
        bc_sb = P["bcs"].tile([128, 512], mybir.dt.bfloat16, tag="bcs", name="bc_mean_sb")[:, :n]
        bc_copy(bc_sb, bc)
        bc2 = P["bc"].tile([128, 512], FP32, tag="bc", name="bc_rstd")[:, :n]
        nc.tensor.matmul(bc2, lhsT=P["ones_row"], rhs=rstd_bf, start=True, stop=True)
        bc2_sb = P["bcs"].tile([128, 512], mybir.dt.bfloat16, tag="bcs", name="bc_rstd_sb")[:, :n]
        bc_copy(bc2_sb, bc2)
        # rounds always split Pool/DVE: all-Pool serializes ~10us and leaves
        # the PE with nothing runnable at the end of the attention phase
        if pool_only:
            eng = [nc.gpsimd, nc.vector, nc.gpsimd, nc.vector]
        else:
            eng = [nc.vector, nc.vector, nc.gpsimd, nc.gpsimd]
        # round 1: subtract broadcast mean (sq_tile becomes the scratch)
        for dt in range(DT):
            eng[dt].tensor_tensor(sq_tile[:, dt, :], resid_view[:, dt, :], bc_sb, op=OP.subtract)
        # round 2: multiply broadcast rstd (in place), then affine via ACT
        for dt in range(DT):
            eng[dt].tensor_tensor(sq_tile[:, dt, :], sq_tile[:, dt, :], bc2_sb, op=OP.mult)
            out_cb(dt, sq_tile[:, dt, :])

    return stage_sq, stage_stats, stage_apply


def _emit_ln_T(nc, P, resid_view, sq_tile, stats_sb, out_cb, n=512, pool_only=False):
    for stage in _ln_stages(nc, P, resid_view, sq_tile, stats_sb, out_cb, n, pool_only):
        stage()


def build_program(has_bv=False):
    nc = bacc_mod.Bacc(None, target_bir_lowering=False)

    qT_d = nc.dram_tensor("qT", [D, WPC], MM_DT, kind="ExternalInput")
    kT_d = nc.dram_tensor("kT", [D, KPC], FP8, kind="ExternalInput")
    # raw v, token-major, phase-grouped: vP[w, f*D + d] = value[4w+f, d]
    vP_d = nc.dram_tensor("vP", [WPC, F * D], MM_DT, kind="ExternalInput")
    wq_d = nc.dram_tensor("w_q", [D, D], MM_DT, kind="ExternalInput")
    wk_d = nc.dram_tensor("w_k", [D, D], FP8, kind="ExternalInput")
    wv_d = nc.dram_tensor("w_v", [D, D], MM_DT, kind="ExternalInput")
    w1_d = nc.dram_tensor("ffn_w1", [D, D], MM_DT, kind="ExternalInput")
    w2_d = nc.dram_tensor("ffn_w2", [D, D], MM_DT, kind="ExternalInput")
    bq_d = nc.dram_tensor("b_q", [D], FP32, kind="ExternalInput")
    bk_d = nc.dram_tensor("b_k", [D], FP32, kind="ExternalInput")
    # v-bias folded pre-projection: u = Wv^-T @ bv, applied as one extra
    # stt step on the raw-v downsample (omitted entirely when bv == 0)
    u_d = nc.dram_tensor("ufold", [D], MM_DT, kind="ExternalInput") if has_bv else None
    b1_d = nc.dram_tensor("ffn_b1", [D], FP32, kind="ExternalInput")
    b2_d = nc.dram_tensor("ffn_b2", [D], FP32, kind="ExternalInput")
    g1_d = nc.dram_tensor("ln1_g", [D], FP32, kind="ExternalInput")
    gb1_d = nc.dram_tensor("ln1_b", [D], FP32, kind="ExternalInput")
    g2_d = nc.dram_tensor("ln2_g", [D], FP32, kind="ExternalInput")
    gb2_d = nc.dram_tensor("ln2_b", [D], FP32, kind="ExternalInput")
    mask_d = nc.dram_tensor("cmask", [128, 512], FP32, kind="ExternalInput")
    ident_d = nc.dram_tensor("cident", [128, 128], MM_DT, kind="ExternalInput")
    outT_d = nc.dram_tensor("outT", [D, WPC], MM_DT, kind="ExternalOutput")

    qT_t = qT_d.rearrange("(o p) n -> p o n", p=128)
    kT_t = kT_d.rearrange("(o p) n -> p o n", p=128)
    vP_t = vP_d.rearrange("(o p) (f d) -> p o f d", p=128, f=F)
    outT_t = outT_d.rearrange("(o p) n -> p o n", p=128)

    with tile.TileContext(nc) as tc, contextlib.ExitStack() as ctx:
        singles = ctx.enter_context(tc.tile_pool(name="singles", bufs=1))
        inp = ctx.enter_context(tc.tile_pool(name="inp", bufs=4))
        ktp_p = ctx.enter_context(tc.tile_pool(name="ktp", bufs=1))
        att_p = ctx.enter_context(tc.tile_pool(name="att", bufs=2))
        resid_p = ctx.enter_context(tc.tile_pool(name="resid", bufs=2))
        hT_p = ctx.enter_context(tc.tile_pool(name="hT", bufs=1))
        out_p = ctx.enter_context(tc.tile_pool(name="outp", bufs=2))
        small = ctx.enter_context(tc.tile_pool(name="small", bufs=1))
        bcs_p = ctx.enter_context(tc.tile_pool(name="bcs", bufs=2))
        ps_proj = ctx.enter_context(tc.tile_pool(name="ps_proj", bufs=3, space="PSUM"))
        ps_tr = ctx.enter_context(tc.tile_pool(name="ps_tr", bufs=2, space="PSUM"))
        ps_st = ctx.enter_context(tc.tile_pool(name="ps_st", bufs=1, space="PSUM"))
        ps_bc = ctx.enter_context(tc.tile_pool(name="ps_bc", bufs=1, space="PSUM"))

        def load_w(d, tg):
            t = singles.tile([128, DT, 512], MM_DT, tag=tg)
            nc.sync.dma_start(out=t, in_=d.rearrange("(o p) n -> p o n", p=128))
            return t

        def load_b(d, tg):
            t = singles.tile([128, DT], FP32, tag=tg)
            nc.sync.dma_start(out=t, in_=d.rearrange("(o p) -> p o", p=128))
            return t

        # issue order matters: Sync issues DMAs in program order, and the PE's
        # first work (q-proj superblock 0) must not wait behind a dozen
        # constant loads — wq and the first q superblock go first, ki-tile
        # interleaved so the do-loop's first matmuls start after ~2 tiles.
        # PE p-state warmup: the first real matmuls otherwise run 2-3x slow
        # while the clock ramps. Grind on a memset tile until data arrives.
        warm = singles.tile([128, 512], MM_DT, tag="warm")
        nc.gpsimd.memset(warm, 0.0)
        warm_c = singles.tile([128, 1], MM_DT, tag="warm_c")
        nc.gpsimd.memset(warm_c, 0.0)
        ps_warm = ps_st.tile([1, 512], FP32, tag="warm_ps", name="warm_ps")
        for _ in range(14):
            nc.tensor.matmul(ps_warm, lhsT=warm_c, rhs=warm, start=True, stop=True)

        # Early loads are spread across engine issue queues: each dma_start
        # costs ~800ns of dispatch on its issuing sequencer, so serializing
        # them all on Sync delays the PE's first work by ~10us.
        # ki-interleaved split loads: the first matmul's dependency is only
        # the first (wq.ki0, q0.ki0) pair, not the whole megabyte
        wq_sb = singles.tile([128, DT, 512], MM_DT, tag="wq")
        q_in0 = inp.tile([128, DT, 512], MM_DT, tag="in_t", name="q_in0")
        wq_r = wq_d.rearrange("(o p) n -> p o n", p=128)
        for ki in range(DT):
            nc.sync.dma_start(out=wq_sb[:, ki, :], in_=wq_r[:, ki, :])
            nc.sync.dma_start(out=q_in0[:, ki, :], in_=qT_t[:, ki, ts(0, 512)])
        k0 = inp.tile([128, DT, 512], FP8, tag="in_k", name="kv0")
        wk_sb = singles.tile([128, DT, 512], FP8, tag="wk")
        wk_r = wk_d.rearrange("(o p) n -> p o n", p=128)
        for ki in range(DT):
            nc.scalar.dma_start(out=wk_sb[:, ki, :], in_=wk_r[:, ki, :])
            nc.scalar.dma_start(out=k0[:, ki, :], in_=kT_t[:, ki, ts(0, 512)])
        q_in1 = inp.tile([128, DT, 512], MM_DT, tag="in_t", name="q_in1")
        nc.sync.dma_start(out=q_in1, in_=qT_t[:, :, ts(1, 512)])
        v0 = inp.tile([128, F, 512], MM_DT, tag="in_t", name="kv0")
        nc.gpsimd.dma_start(out=v0, in_=vP_t[:, 0, :, :])
        kv0 = [k0, v0]

        def load_b_q(d, tg, q):
            t = singles.tile([128, DT], FP32, tag=tg)
            q.dma_start(out=t, in_=d.rearrange("(o p) -> p o", p=128))
            return t

        bq_sb = load_b_q(bq_d, "bq", nc.gpsimd)
        bk_sb = load_b_q(bk_d, "bk", nc.gpsimd)
        wv_sb = load_w(wv_d, "wv")
        mask = singles.tile([128, 512], FP32, tag="mask")
        nc.gpsimd.dma_start(out=mask, in_=mask_d[:, :])
        if has_bv:
            u_rep = singles.tile([128, 512], MM_DT, tag="u_rep")
            nc.gpsimd.dma_start(
                out=u_rep, in_=bass.AP(tensor=u_d, offset=0, ap=[[0, 128], [1, 512]])
            )
        identity = singles.tile([128, 128], MM_DT, tag="ident")
        nc.gpsimd.dma_start(out=identity, in_=ident_d[:, :])
        g1_sb = load_b_q(g1_d, "g1", nc.gpsimd)
        gb1_sb = load_b_q(gb1_d, "gb1", nc.gpsimd)
        ones_col = singles.tile([128, 1], MM_DT, tag="ones_col")
        nc.gpsimd.memset(ones_col, 1.0)
        ones_row = singles.tile([1, 128], MM_DT, tag="ones_row")
        nc.gpsimd.memset(ones_row, 1.0)
        eps_t = singles.tile([1, 1], FP32, tag="eps")
        nc.gpsimd.memset(eps_t, EPS)
        late = {}

        def load_late_consts():
            late["w1"] = load_w(w1_d, "w1")
            late["b1"] = load_b(b1_d, "b1")
            late["w2"] = load_w(w2_d, "w2")
            late["b2"] = load_b(b2_d, "b2")
            late["g2"] = load_b(g2_d, "g2")
            late["gb2"] = load_b(gb2_d, "gb2")

        P = {
            "st": ps_st, "bc": ps_bc, "bcs": bcs_p, "small": small,
            "ones_col": ones_col, "ones_row": ones_row, "eps_t": eps_t,
        }

        qTp = singles.tile([128, DT, WPC], MM_DT, tag="qTp")
        xT = singles.tile([128, DT, WPC], MM_DT, tag="xT")

        def proj_T(w_sb, bias_sb, in_sb, out_sb, out_col0, n):
            for do in range(DT):
                ps = ps_proj.tile([128, 512], FP32, tag="proj_ps", name="proj_ps")
                ps = ps[:, :n]
                for ki in range(DT):
                    nc.tensor.matmul(
                        ps, lhsT=w_sb[:, ki, ts(do, 128)], rhs=in_sb[:, ki, :n],
                        start=(ki == 0), stop=(ki == DT - 1),
                    )
                nc.scalar.activation(
                    out=out_sb[:, do, ds(out_col0, n)], in_=ps, func=AF.Relu,
                    bias=bias_sb[:, do : do + 1], scale=1.0,
                )

        # ---- phase 1: q projection ----
        for blk, q_in in ((0, q_in0), (1, q_in1)):
            proj_T(wq_sb, bq_sb, q_in, qTp, blk * 512, 512)

        # ---- phase 2: attention ----
        residT = {}  # superblock -> tile [128, DT, 512]
        aoT = {}  # superblock -> weighted raw-v downsample, transposed, bf16

        def emit_kproj(b):
            if b == 0:
                k_in, v_in = kv0
            else:
                k_in = inp.tile([128, DT, 512], FP8, tag="in_k")
                nc.sync.dma_start(out=k_in, in_=kT_t[:, :, ts(b, 512)])
                v_in = inp.tile([128, F, 512], MM_DT, tag="in_t")
                nc.sync.dma_start(out=v_in, in_=vP_t[:, b, :, :])

            # fp8 DoubleRow kproj: each matmul consumes two 128-row k-tiles
            kTp = ktp_p.tile([128, DT, 512], MM_DT, tag="kTp")
            for do in range(DT):
                ps = ps_proj.tile([128, 512], FP32, tag="proj_ps", name="kproj_ps")
                for j in range(2):
                    nc.tensor.matmul(
                        ps,
                        lhsT=wk_sb[:, 2 * j : 2 * j + 2, ts(do, 128)],
                        rhs=k_in[:, 2 * j : 2 * j + 2, :],
                        start=(j == 0), stop=(j == 1),
                        perf_mode=mybir.MatmulPerfMode.DoubleRow,
                    )
                nc.scalar.activation(
                    out=kTp[:, do, :], in_=ps, func=AF.Relu,
                    bias=bk_sb[:, do : do + 1], scale=1.0,
                )
            return kTp, v_in

        def emit_scores(b, kTp, v_in):
            sc_ps = ps_proj.tile([128, 512], FP32, tag="proj_ps", name="sc_ps")
            for ki in range(DT):
                nc.tensor.matmul(
                    sc_ps, lhsT=qTp[:, ki, ts(b, 128)], rhs=kTp[:, ki, :],
                    start=(ki == 0), stop=(ki == DT - 1),
                )
            sm = att_p.tile([128, 512], FP32, tag="sm")
            nc.vector.tensor_tensor(sm, sc_ps, mask, op=OP.mult)
            wts = small.tile([128, F], FP32, tag="wts")
            nc.vector.tensor_reduce(
                out=wts, in_=sm.rearrange("p (kw f) -> p f kw", f=F),
                axis=mybir.AxisListType.X, op=OP.add,
            )
            # weighted downsample of RAW v (linear projection commuted after)
            acc = att_p.tile([128, 512], MM_DT, tag="ao_acc", name="ao_acc0")
            nc.vector.tensor_scalar(
                out=acc, in0=v_in[:, 0, :], scalar1=wts[:, 0:1], scalar2=None,
                op0=OP.mult,
            )
            chain = list(range(1, F)) + (["bias"] if has_bv else [])
            for f in chain:
                tg = "ao_final" if f == chain[-1] else "ao_acc"
                nxt = att_p.tile([128, 512], MM_DT, tag=tg, name="ao_acc")
                if f == "bias":
                    ws_f = small.tile([128, 1], FP32, tag="ws_f", name="ws_f")
                    nc.vector.tensor_reduce(
                        out=ws_f, in_=wts, axis=mybir.AxisListType.X, op=OP.add
                    )
                    nc.vector.scalar_tensor_tensor(
                        out=nxt, in0=u_rep, scalar=ws_f[:, 0:1], in1=acc,
                        op0=OP.mult, op1=OP.add,
                    )
                else:
                    nc.vector.scalar_tensor_tensor(
                        out=nxt, in0=v_in[:, f, :], scalar=wts[:, f : f + 1], in1=acc,
                        op0=OP.mult, op1=OP.add,
                    )
                acc = nxt
            return acc

        def emit_mid(b, acc):
            sb, col = b // 4, (b % 4) * 128
            if col == 0:
                aoT[sb] = resid_p.tile([128, DT, 512], MM_DT, tag="aoT", name="aoT")
            a = aoT[sb]
            ps_t = ps_tr.tile([128, 512], MM_DT, tag="tr_ps", name="tr_ps")
            for j in range(DT):
                nc.tensor.transpose(ps_t[:, ts(j, 128)], acc[:, ts(j, 128)], identity)
            nc.scalar.activation(
                out=a[:, :, ds(col, 128)],
                in_=ps_t.rearrange("p (j n) -> p j n", j=DT),
                func=AF.Copy,
            )

        def emit_vproj(sb):
            # project the downsampled values: out = aoT @ Wv + qTp (identity-
            # matmul step: the residual add costs one more PE accumulation
            # instead of a DVE op on the PSUM)
            r = resid_p.tile([128, DT, 512], MM_DT, tag="residT", name="residT")
            residT[sb] = r
            for do in range(DT):
                ps = ps_proj.tile([128, 512], FP32, tag="proj_ps", name="vds_ps")
                for ki in range(DT):
                    nc.tensor.matmul(
                        ps, lhsT=wv_sb[:, ki, ts(do, 128)], rhs=aoT[sb][:, ki, :],
                        start=(ki == 0), stop=False,
                    )
                nc.tensor.matmul(
                    ps, lhsT=identity, rhs=qTp[:, do, ts(sb, 512)],
                    start=False, stop=True,
                )
                nc.scalar.activation(out=r[:, do, :], in_=ps, func=AF.Copy)

        def emit_ln1(sb):
            sq = resid_p.tile([128, DT, 512], MM_DT, tag="sq1")
            stats = small.tile([1, 1024], MM_DT, tag="stats1")

            def write_x(dt, src, sb=sb):
                if sb == 0:
                    # affine on DVE (2-scalar tensor_scalar) - ACT is the
                    # hottest engine in this window
                    nc.vector.tensor_scalar(
                        out=xT[:, dt, ts(sb, 512)], in0=src,
                        scalar1=g1_sb[:, dt : dt + 1], scalar2=gb1_sb[:, dt : dt + 1],
                        op0=OP.mult, op1=OP.add,
                    )
                else:
                    nc.scalar.activation(
                        out=xT[:, dt, ts(sb, 512)], in_=src, func=AF.Identity,
                        bias=gb1_sb[:, dt : dt + 1], scale=g1_sb[:, dt : dt + 1],
                    )

            return _ln_stages(
                nc, P, residT[sb][:], sq, stats, write_x, pool_only=(sb == 0)
            )

        def emit_ffn(blk, c0=0, n=512):
            col = blk * 512 + c0
            hT = hT_p.tile([128, DT, 512], MM_DT, tag="hT")
            for ht in range(DT):
                ps = ps_proj.tile([128, 512], FP32, tag="proj_ps", name="ffn1_ps")
                for ki in range(DT):
                    nc.tensor.matmul(
                        ps[:, :n], lhsT=late["w1"][:, ki, ts(ht, 128)],
                        rhs=xT[:, ki, ds(col, n)],
                        start=(ki == 0), stop=(ki == DT - 1),
                    )
                nc.scalar.activation(
                    out=hT[:, ht, :n], in_=ps[:, :n], func=AF.Relu,
                    bias=late["b1"][:, ht : ht + 1], scale=1.0,
                )
            resid2 = resid_p.tile([128, DT, 512], MM_DT, tag="resid2")
            for dt in range(DT):
                ps = ps_proj.tile([128, 512], FP32, tag="proj_ps", name="ffn2_ps")
                for hi in range(DT):
                    nc.tensor.matmul(
                        ps[:, :n], lhsT=late["w2"][:, hi, ts(dt, 128)],
                        rhs=hT[:, hi, :n],
                        start=(hi == 0), stop=(hi == DT - 1),
                    )
                nc.vector.scalar_tensor_tensor(
                    out=resid2[:, dt, :n], in0=ps[:, :n], scalar=late["b2"][:, dt : dt + 1],
                    in1=xT[:, dt, ds(col, n)], op0=OP.add, op1=OP.add,
                )
            sq2 = hT_p.tile([128, DT, 512], MM_DT, tag="sq2")
            stats2 = small.tile([1, 1024], MM_DT, tag="stats2")
            out_sb = out_p.tile([128, DT, 512], MM_DT, tag="out_sb")

            def write_out(dt, src, out_sb=out_sb, col=col, n=n):
                nc.scalar.activation(
                    out=out_sb[:, dt, :n], in_=src, func=AF.Identity,
                    bias=late["gb2"][:, dt : dt + 1], scale=late["g2"][:, dt : dt + 1],
                )
                # per-dt chunked writeback overlaps the remaining apply work
                nc.sync.dma_start(out=outT_t[:, dt, ds(col, n)], in_=out_sb[:, dt, :n])

            _emit_ln_T(nc, P, resid2[:, :, :n], sq2[:, :, :n], stats2, write_out, n=n)

        # mids lag TWO blocks: block b-2's stt chain (4us of serial DVE) is
        # guaranteed drained, so the transposes never stall the PE
        accs = {}
        for b in range(NBLK):
            kv = emit_kproj(b)
            if b == 0:
                load_late_consts()
            if b >= 2:
                emit_mid(b - 2, accs.pop(b - 2))
            accs[b] = emit_scores(b, *kv)
            # LN1(0) staged across iterations 5-7: each stage's PE
            # instructions only enter the stream after their Pool/ACT input
            # chain has had a full block of slack, so the PE never stalls
            if b == 5:
                emit_vproj(0)
                ln1_stages = emit_ln1(0)
                ln1_stages[0]()  # squares (Pool/DVE only, no PE)
            if b == 6:
                ln1_stages[1]()  # stats matmuls (squares have drained)
            if b == 7:
                ln1_stages[2]()  # broadcast + apply rounds
                # FFN(0) right behind: its matmuls consume xT d-tiles as the
                # apply emits them, filling the PE while blocks 6-7's DVE
                # chains drain
                emit_ffn(0)
        emit_mid(NBLK - 2, accs.pop(NBLK - 2))
        emit_mid(NBLK - 1, accs.pop(NBLK - 1))
        emit_vproj(NSB - 1)
        for _stage in emit_ln1(NSB - 1):
            _stage()
        emit_ffn(NSB - 1)

    nc.finalize()
    return nc


def kernel(**inputs):
    has_bv = bool(np.any(np.asarray(inputs["b_v"], dtype=np.float32) != 0.0))
    key = ("prog", has_bv)
    if key not in _CACHE:
        _CACHE[key] = build_program(has_bv)
    nc = _CACHE[key]

    import ml_dtypes

    f32 = lambda x: np.ascontiguousarray(np.asarray(x), dtype=np.float32)
    bf16 = lambda x: np.ascontiguousarray(np.asarray(x, dtype=np.float32).astype(ml_dtypes.bfloat16))
    fp8 = lambda x: np.ascontiguousarray(np.asarray(x, dtype=np.float32).astype(ml_dtypes.float8_e4m3))
    query, key_, value = f32(inputs["query"]), f32(inputs["key"]), f32(inputs["value"])

    shared = {
        n: f32(inputs[n])
        for n in ("b_q", "b_k", "ffn_b1", "ffn_b2",
                  "ln1_g", "ln1_b", "ln2_g", "ln2_b")
    }
    for n in ("w_q", "w_v", "ffn_w1", "ffn_w2"):
        shared[n] = bf16(inputs[n])
    shared["w_k"] = fp8(inputs["w_k"])
    if has_bv:
        wv64 = np.asarray(inputs["w_v"], dtype=np.float64)
        u = np.linalg.solve(wv64.T, np.asarray(inputs["b_v"], dtype=np.float64))
        shared["ufold"] = bf16(u)
    p_idx = np.arange(128)[:, None]
    k_idx = np.arange(512)[None, :]
    shared["cmask"] = ((k_idx - 4 * p_idx >= 0) & (k_idx - 4 * p_idx <= 3)).astype(np.float32)
    shared["cident"] = bf16(np.eye(128))

    in_maps = []
    for c in range(NCORES):
        bi, half = c // 2, c % 2
        w0 = half * WPC
        m = dict(shared)
        m["qT"] = bf16(query[bi, w0 : w0 + WPC, :].T)
        m["kT"] = fp8(key_[bi, w0 * F : (w0 + WPC) * F, :].T)
        m["vP"] = bf16(value[bi, w0 * F : (w0 + WPC) * F, :].reshape(WPC, F * D))
        in_maps.append(m)

    res = run_bass_kernel_spmd(nc, in_maps, core_ids=list(range(NCORES)))
    _CACHE["last_result"] = res
    out = np.empty((B, SQ, D), dtype=np.float32)
    for c in range(NCORES):
        bi, half = c // 2, c % 2
        w0 = half * WPC
        out[bi, w0 : w0 + WPC, :] = res.results[c]["outT"].T.astype(np.float32)
    return out



# revision 105
# speedup vs baseline: 1.0324x; 1.0324x over previous
"""Trainium2 Bass kernel for nn_AttentionSampling (sparse window attention block).

Sharding: 8 cores, data-parallel, 1024 windows (half a batch) per core; windows are
independent so there is no cross-core communication. Activations live in a transposed
[d, tokens] layout (host pre-transposes q/k) so every projection runs with the
weight stationary; raw v ships token-major phase-grouped. Matmul operands are bf16
(k-projection fp8 e4m3); attention weights and LN statistics math stay fp32.

Key structural points:
- v-projection is LINEAR, so the weighted window-sum commutes before it:
  the kernel reduces 4096 raw v tokens -> 1024 on the DVE (stt chain with
  per-window band-masked score weights), transposes the downsample on the
  PE, and projects only 1024 tokens. The (zero here) v-bias folds in as
  u = Wv^-T bv pre-projection when nonzero.
- The q-residual add rides the v-projection PSUM as an identity-matmul
  accumulation step.
- Band-masked scores + strided reduction extract the 4 per-window dot
  products as per-partition scalars.
- LayerNorm runs in the transposed domain with bf16 stats/broadcast
  matmuls (4x cheaper than fp32 on the PE); apply rounds are spread
  across DVE and the Pool engine; LN1(0), which overlaps the attention
  phase, runs entirely off the hot ACT/DVE pair.
- Block mids (PE transposes of the downsample) lag two blocks so the
  serial DVE stt chain can never stall the PE.
- Head: ki-interleaved first loads spread over three DMA issue queues +
  PE p-state warmup matmuls. Tail: per-dt chunked bf16 output DMA.
"""

import sys
import types

# If BASS_TRACE is set in an environment whose antenv package lacks
# axon_hooks, run_bass_kernel_spmd would crash on import; provide a stub
# (a None hook makes bass_utils skip tracing gracefully).
try:
    import antenv.axon_hooks  # noqa: F401
except ImportError:
    _m = types.ModuleType("antenv.axon_hooks")
    _m.get_axon_ntff_profile_hook = lambda: None
    _m.set_axon_ntff_profile_hook = lambda h: None
    sys.modules["antenv.axon_hooks"] = _m
    try:
        import antenv

        antenv.axon_hooks = _m
    except ImportError:
        pass

import contextlib

import numpy as np

import concourse.bass as bass
import concourse.bacc as bacc_mod
import concourse.mybir as mybir
import concourse.tile as tile
from concourse.bass import ts, ds
from concourse.bass_utils import run_bass_kernel_spmd

FP32 = mybir.dt.float32
FP32R = mybir.dt.float32r
AF = mybir.ActivationFunctionType
OP = mybir.AluOpType

MM_DT = mybir.dt.bfloat16  # matmul operands; attention weights/LN stay fp32
FP8 = mybir.dt.float8e4    # k-projection operands (DoubleRow perf mode)

B, SQ, SK, D, F = 4, 2048, 8192, 512, 4
NCORES = 8
WPC = B * SQ // NCORES        # 1024 windows (= tokens) per core
KPC = WPC * F                 # 4096 keys per core
NBLK = WPC // 128             # 8 attention blocks: 128 windows / 512 keys
NSB = WPC // 512              # 2 superblocks of 512 tokens
DT = D // 128                 # 4 d-tiles
EPS = 1e-5

_CACHE = {}


def _emit_ln_T(nc, P, resid_view, sq_tile, stats_sb, out_cb, n=512, pool_only=False):
    """Transposed LayerNorm over D for an n-token block.

    resid_view/sq_tile: [128, DT, n] bf16; sq_tile doubles as apply scratch.
    stats_sb: [1, 2n] bf16 (mean | rstd) — rhs of the broadcast matmuls.
    out_cb(dt, src): write normalized+affine output for d-tile dt from src.

    All PE matmuls here use bf16 operands (1 cycle/row vs 4 for fp32).
    """
    mean_bf = stats_sb[:, :n]
    rstd_bf = stats_sb[:, n : 2 * n]

    # sq per d-tile, alternating engines: the stats accumulation consumes
    # sq[dt] in order, so early tiles unblock the PE chain sooner
    sq_engines = [nc.gpsimd] * 4 if pool_only else [nc.vector, nc.gpsimd, nc.vector, nc.gpsimd]
    for dt in range(DT):
        sq_engines[dt].tensor_tensor(
            sq_tile[:, dt, :], resid_view[:, dt, :], resid_view[:, dt, :], op=OP.mult
        )

    st_sum = P["st"].tile([1, 512], FP32, tag="st", name="st_sum")[:, :n]
    for dt in range(DT):
        nc.tensor.matmul(
            st_sum, lhsT=P["ones_col"], rhs=resid_view[:, dt, :],
            start=(dt == 0), stop=(dt == DT - 1),
        )
    nc.scalar.activation(out=mean_bf, in_=st_sum, func=AF.Copy, scale=1.0 / D)

    st_sq = P["st"].tile([1, 512], FP32, tag="st", name="st_sq")[:, :n]
    for dt in range(DT):
        nc.tensor.matmul(
            st_sq, lhsT=P["ones_col"], rhs=sq_tile[:, dt, :],
            start=(dt == 0), stop=(dt == DT - 1),
        )
    e2 = P["small"].tile([1, 512], FP32, tag="e2", name="e2")[:, :n]
    var = P["small"].tile([1, 512], FP32, tag="var", name="var")[:, :n]
    nc.scalar.activation(out=e2, in_=st_sq, func=AF.Copy, scale=1.0 / D)
    nc.vector.tensor_tensor(var, mean_bf, mean_bf, op=OP.mult)
    nc.vector.tensor_tensor(var, e2, var, op=OP.subtract)
    rstd = P["small"].tile([1, 512], FP32, tag="rstd", name="rstd")[:, :n]
    nc.scalar.activation(out=var, in_=var, func=AF.Sqrt, bias=P["eps_t"], scale=1.0)
    nc.vector.reciprocal_approx_fast(out=rstd, in_=var)
    nc.scalar.activation(out=rstd_bf, in_=rstd, func=AF.Copy)

    # broadcast mean/rstd across partitions via PE rank-1, then copy to SBUF
    # bf16 so the apply rounds can run on the Pool engine (no PSUM access)
    # bc copies go to DVE when the LN runs in the ACT-hot attention window
    def bc_copy(dst, src):
        if pool_only:
            nc.vector.tensor_scalar(out=dst, in0=src, scalar1=1.0, scalar2=None, op0=OP.mult)
        else:
            nc.scalar.activation(out=dst, in_=src, func=AF.Copy)

    bc = P["bc"].tile([128, 512], FP32, tag="bc", name="bc_mean")[:, :n]
    nc.tensor.matmul(bc, lhsT=P["ones_row"], rhs=mean_bf, start=True, stop=True)
    bc_sb = P["bcs"].tile([128, 512], mybir.dt.bfloat16, tag="bcs", name="bc_mean_sb")[:, :n]
    bc_copy(bc_sb, bc)
    bc2 = P["bc"].tile([128, 512], FP32, tag="bc", name="bc_rstd")[:, :n]
    nc.tensor.matmul(bc2, lhsT=P["ones_row"], rhs=rstd_bf, start=True, stop=True)
    bc2_sb = P["bcs"].tile([128, 512], mybir.dt.bfloat16, tag="bcs", name="bc_rstd_sb")[:, :n]
    bc_copy(bc2_sb, bc2)
    # apply rounds: Pool-only keeps the whole apply off the congested DVE
    # (for LN1, which overlaps the attention phase); the tail LNs split
    # DVE/Pool since there latency matters and DVE has drained
    if pool_only:
        eng = [nc.gpsimd] * 4
    else:
        eng = [nc.vector, nc.vector, nc.gpsimd, nc.gpsimd]
    # round 1: subtract broadcast mean (sq_tile becomes the scratch)
    for dt in range(DT):
        eng[dt].tensor_tensor(sq_tile[:, dt, :], resid_view[:, dt, :], bc_sb, op=OP.subtract)
    # round 2: multiply broadcast rstd (in place), then affine via ACT
    for dt in range(DT):
        eng[dt].tensor_tensor(sq_tile[:, dt, :], sq_tile[:, dt, :], bc2_sb, op=OP.mult)
        out_cb(dt, sq_tile[:, dt, :])


def build_program(has_bv=False):
    nc = bacc_mod.Bacc(None, target_bir_lowering=False)

    qT_d = nc.dram_tensor("qT", [D, WPC], MM_DT, kind="ExternalInput")
    kT_d = nc.dram_tensor("kT", [D, KPC], FP8, kind="ExternalInput")
    # raw v, token-major, phase-grouped: vP[w, f*D + d] = value[4w+f, d]
    vP_d = nc.dram_tensor("vP", [WPC, F * D], MM_DT, kind="ExternalInput")
    wq_d = nc.dram_tensor("w_q", [D, D], MM_DT, kind="ExternalInput")
    wk_d = nc.dram_tensor("w_k", [D, D], FP8, kind="ExternalInput")
    wv_d = nc.dram_tensor("w_v", [D, D], MM_DT, kind="ExternalInput")
    w1_d = nc.dram_tensor("ffn_w1", [D, D], MM_DT, kind="ExternalInput")
    w2_d = nc.dram_tensor("ffn_w2", [D, D], MM_DT, kind="ExternalInput")
    bq_d = nc.dram_tensor("b_q", [D], FP32, kind="ExternalInput")
    bk_d = nc.dram_tensor("b_k", [D], FP32, kind="ExternalInput")
    # v-bias folded pre-projection: u = Wv^-T @ bv, applied as one extra
    # stt step on the raw-v downsample (omitted entirely when bv == 0)
    u_d = nc.dram_tensor("ufold", [D], MM_DT, kind="ExternalInput") if has_bv else None
    b1_d = nc.dram_tensor("ffn_b1", [D], FP32, kind="ExternalInput")
    b2_d = nc.dram_tensor("ffn_b2", [D], FP32, kind="ExternalInput")
    g1_d = nc.dram_tensor("ln1_g", [D], FP32, kind="ExternalInput")
    gb1_d = nc.dram_tensor("ln1_b", [D], FP32, kind="ExternalInput")
    g2_d = nc.dram_tensor("ln2_g", [D], FP32, kind="ExternalInput")
    gb2_d = nc.dram_tensor("ln2_b", [D], FP32, kind="ExternalInput")
    mask_d = nc.dram_tensor("cmask", [128, 512], FP32, kind="ExternalInput")
    ident_d = nc.dram_tensor("cident", [128, 128], MM_DT, kind="ExternalInput")
    outT_d = nc.dram_tensor("outT", [D, WPC], MM_DT, kind="ExternalOutput")

    qT_t = qT_d.rearrange("(o p) n -> p o n", p=128)
    kT_t = kT_d.rearrange("(o p) n -> p o n", p=128)
    vP_t = vP_d.rearrange("(o p) (f d) -> p o f d", p=128, f=F)
    outT_t = outT_d.rearrange("(o p) n -> p o n", p=128)

    with tile.TileContext(nc) as tc, contextlib.ExitStack() as ctx:
        singles = ctx.enter_context(tc.tile_pool(name="singles", bufs=1))
        inp = ctx.enter_context(tc.tile_pool(name="inp", bufs=4))
        # 2 kTp buffers: block b+1's RELU evictions would otherwise wait on
        # block b's score matmuls still reading the single buffer (WAR)
        ktp_p = ctx.enter_context(tc.tile_pool(name="ktp", bufs=2))
        att_p = ctx.enter_context(tc.tile_pool(name="att", bufs=2))
        resid_p = ctx.enter_context(tc.tile_pool(name="resid", bufs=2))
        hT_p = ctx.enter_context(tc.tile_pool(name="hT", bufs=1))
        out_p = ctx.enter_context(tc.tile_pool(name="outp", bufs=2))
        small = ctx.enter_context(tc.tile_pool(name="small", bufs=1))
        bcs_p = ctx.enter_context(tc.tile_pool(name="bcs", bufs=2))
        ps_proj = ctx.enter_context(tc.tile_pool(name="ps_proj", bufs=3, space="PSUM"))
        ps_tr = ctx.enter_context(tc.tile_pool(name="ps_tr", bufs=2, space="PSUM"))
        ps_st = ctx.enter_context(tc.tile_pool(name="ps_st", bufs=1, space="PSUM"))
        ps_bc = ctx.enter_context(tc.tile_pool(name="ps_bc", bufs=1, space="PSUM"))

        def load_w(d, tg):
            t = singles.tile([128, DT, 512], MM_DT, tag=tg)
            nc.sync.dma_start(out=t, in_=d.rearrange("(o p) n -> p o n", p=128))
            return t

        def load_b(d, tg):
            t = singles.tile([128, DT], FP32, tag=tg)
            nc.sync.dma_start(out=t, in_=d.rearrange("(o p) -> p o", p=128))
            return t

        # issue order matters: Sync issues DMAs in program order, and the PE's
        # first work (q-proj superblock 0) must not wait behind a dozen
        # constant loads — wq and the first q superblock go first, ki-tile
        # interleaved so the do-loop's first matmuls start after ~2 tiles.
        # PE p-state warmup: the first real matmuls otherwise run 2-3x slow
        # while the clock ramps. Grind on a memset tile until data arrives.
        warm = singles.tile([128, 512], MM_DT, tag="warm")
        nc.gpsimd.memset(warm, 0.0)
        warm_c = singles.tile([128, 1], MM_DT, tag="warm_c")
        nc.gpsimd.memset(warm_c, 0.0)
        ps_warm = ps_st.tile([1, 512], FP32, tag="warm_ps", name="warm_ps")
        for _ in range(14):
            nc.tensor.matmul(ps_warm, lhsT=warm_c, rhs=warm, start=True, stop=True)

        # Early loads are spread across engine issue queues: each dma_start
        # costs ~800ns of dispatch on its issuing sequencer, so serializing
        # them all on Sync delays the PE's first work by ~10us.
        # ki-interleaved split loads: the first matmul's dependency is only
        # the first (wq.ki0, q0.ki0) pair, not the whole megabyte
        wq_sb = singles.tile([128, DT, 512], MM_DT, tag="wq")
        q_in0 = inp.tile([128, DT, 512], MM_DT, tag="in_t", name="q_in0")
        wq_r = wq_d.rearrange("(o p) n -> p o n", p=128)
        for ki in range(DT):
            nc.sync.dma_start(out=wq_sb[:, ki, :], in_=wq_r[:, ki, :])
            nc.sync.dma_start(out=q_in0[:, ki, :], in_=qT_t[:, ki, ts(0, 512)])
        k0 = inp.tile([128, DT, 512], FP8, tag="in_k", name="kv0")
        wk_sb = singles.tile([128, DT, 512], FP8, tag="wk")
        wk_r = wk_d.rearrange("(o p) n -> p o n", p=128)
        for ki in range(DT):
            nc.scalar.dma_start(out=wk_sb[:, ki, :], in_=wk_r[:, ki, :])
            nc.scalar.dma_start(out=k0[:, ki, :], in_=kT_t[:, ki, ts(0, 512)])
        q_in1 = inp.tile([128, DT, 512], MM_DT, tag="in_t", name="q_in1")
        nc.sync.dma_start(out=q_in1, in_=qT_t[:, :, ts(1, 512)])
        v0 = inp.tile([128, F, 512], MM_DT, tag="in_t", name="kv0")
        nc.gpsimd.dma_start(out=v0, in_=vP_t[:, 0, :, :])
        kv0 = [k0, v0]

        def load_b_q(d, tg, q):
            t = singles.tile([128, DT], FP32, tag=tg)
            q.dma_start(out=t, in_=d.rearrange("(o p) -> p o", p=128))
            return t

        bq_sb = load_b_q(bq_d, "bq", nc.gpsimd)
        bk_sb = load_b_q(bk_d, "bk", nc.gpsimd)
        wv_sb = load_w(wv_d, "wv")
        mask = singles.tile([128, 512], FP32, tag="mask")
        nc.gpsimd.dma_start(out=mask, in_=mask_d[:, :])
        if has_bv:
            u_rep = singles.tile([128, 512], MM_DT, tag="u_rep")
            nc.gpsimd.dma_start(
                out=u_rep, in_=bass.AP(tensor=u_d, offset=0, ap=[[0, 128], [1, 512]])
            )
        identity = singles.tile([128, 128], MM_DT, tag="ident")
        nc.gpsimd.dma_start(out=identity, in_=ident_d[:, :])
        g1_sb = load_b_q(g1_d, "g1", nc.gpsimd)
        gb1_sb = load_b_q(gb1_d, "gb1", nc.gpsimd)
        ones_col = singles.tile([128, 1], MM_DT, tag="ones_col")
        nc.gpsimd.memset(ones_col, 1.0)
        ones_row = singles.tile([1, 128], MM_DT, tag="ones_row")
        nc.gpsimd.memset(ones_row, 1.0)
        eps_t = singles.tile([1, 1], FP32, tag="eps")
        nc.gpsimd.memset(eps_t, EPS)
        late = {}

        def load_late_consts():
            late["w1"] = load_w(w1_d, "w1")
            late["b1"] = load_b(b1_d, "b1")
            late["w2"] = load_w(w2_d, "w2")
            late["b2"] = load_b(b2_d, "b2")
            late["g2"] = load_b(g2_d, "g2")
            late["gb2"] = load_b(gb2_d, "gb2")

        P = {
            "st": ps_st, "bc": ps_bc, "bcs": bcs_p, "small": small,
            "ones_col": ones_col, "ones_row": ones_row, "eps_t": eps_t,
        }

        qTp = singles.tile([128, DT, WPC], MM_DT, tag="qTp")
        xT = singles.tile([128, DT, WPC], MM_DT, tag="xT")

        def proj_T(w_sb, bias_sb, in_sb, out_sb, out_col0, n):
            for do in range(DT):
                ps = ps_proj.tile([128, 512], FP32, tag="proj_ps", name="proj_ps")
                ps = ps[:, :n]
                for ki in range(DT):
                    nc.tensor.matmul(
                        ps, lhsT=w_sb[:, ki, ts(do, 128)], rhs=in_sb[:, ki, :n],
                        start=(ki == 0), stop=(ki == DT - 1),
                    )
                nc.scalar.activation(
                    out=out_sb[:, do, ds(out_col0, n)], in_=ps, func=AF.Relu,
                    bias=bias_sb[:, do : do + 1], scale=1.0,
                )

        # ---- phase 1: q projection ----
        for blk, q_in in ((0, q_in0), (1, q_in1)):
            proj_T(wq_sb, bq_sb, q_in, qTp, blk * 512, 512)

        # ---- phase 2: attention ----
        residT = {}  # superblock -> tile [128, DT, 512]
        aoT = {}  # superblock -> weighted raw-v downsample, transposed, bf16

        def emit_kproj(b):
            if b == 0:
                k_in, v_in = kv0
            else:
                k_in = inp.tile([128, DT, 512], FP8, tag="in_k")
                nc.sync.dma_start(out=k_in, in_=kT_t[:, :, ts(b, 512)])
                v_in = inp.tile([128, F, 512], MM_DT, tag="in_t")
                nc.sync.dma_start(out=v_in, in_=vP_t[:, b, :, :])

            # fp8 DoubleRow kproj: each matmul consumes two 128-row k-tiles
            kTp = ktp_p.tile([128, DT, 512], MM_DT, tag="kTp")
            for do in range(DT):
                ps = ps_proj.tile([128, 512], FP32, tag="proj_ps", name="kproj_ps")
                for j in range(2):
                    nc.tensor.matmul(
                        ps,
                        lhsT=wk_sb[:, 2 * j : 2 * j + 2, ts(do, 128)],
                        rhs=k_in[:, 2 * j : 2 * j + 2, :],
                        start=(j == 0), stop=(j == 1),
                        perf_mode=mybir.MatmulPerfMode.DoubleRow,
                    )
                nc.scalar.activation(
                    out=kTp[:, do, :], in_=ps, func=AF.Relu,
                    bias=bk_sb[:, do : do + 1], scale=1.0,
                )
            return kTp, v_in

        def emit_scores(b, kTp, v_in):
            sc_ps = ps_proj.tile([128, 512], FP32, tag="proj_ps", name="sc_ps")
            for ki in range(DT):
                nc.tensor.matmul(
                    sc_ps, lhsT=qTp[:, ki, ts(b, 128)], rhs=kTp[:, ki, :],
                    start=(ki == 0), stop=(ki == DT - 1),
                )
            sm = att_p.tile([128, 512], FP32, tag="sm")
            nc.vector.tensor_tensor(sm, sc_ps, mask, op=OP.mult)
            wts = small.tile([128, F], FP32, tag="wts")
            nc.vector.tensor_reduce(
                out=wts, in_=sm.rearrange("p (kw f) -> p f kw", f=F),
                axis=mybir.AxisListType.X, op=OP.add,
            )
            # weighted downsample of RAW v (linear projection commuted after)
            acc = att_p.tile([128, 512], MM_DT, tag="ao_acc", name="ao_acc0")
            nc.vector.tensor_scalar(
                out=acc, in0=v_in[:, 0, :], scalar1=wts[:, 0:1], scalar2=None,
                op0=OP.mult,
            )
            chain = list(range(1, F)) + (["bias"] if has_bv else [])
            for f in chain:
                tg = "ao_final" if f == chain[-1] else "ao_acc"
                nxt = att_p.tile([128, 512], MM_DT, tag=tg, name="ao_acc")
                if f == "bias":
                    ws_f = small.tile([128, 1], FP32, tag="ws_f", name="ws_f")
                    nc.vector.tensor_reduce(
                        out=ws_f, in_=wts, axis=mybir.AxisListType.X, op=OP.add
                    )
                    nc.vector.scalar_tensor_tensor(
                        out=nxt, in0=u_rep, scalar=ws_f[:, 0:1], in1=acc,
                        op0=OP.mult, op1=OP.add,
                    )
                else:
                    nc.vector.scalar_tensor_tensor(
                        out=nxt, in0=v_in[:, f, :], scalar=wts[:, f : f + 1], in1=acc,
                        op0=OP.mult, op1=OP.add,
                    )
                acc = nxt
            return acc

        def emit_mid(b, acc):
            sb, col = b // 4, (b % 4) * 128
            if col == 0:
                aoT[sb] = resid_p.tile([128, DT, 512], MM_DT, tag="aoT", name="aoT")
            a = aoT[sb]
            ps_t = ps_tr.tile([128, 512], MM_DT, tag="tr_ps", name="tr_ps")
            for j in range(DT):
                nc.tensor.transpose(ps_t[:, ts(j, 128)], acc[:, ts(j, 128)], identity)
            nc.scalar.activation(
                out=a[:, :, ds(col, 128)],
                in_=ps_t.rearrange("p (j n) -> p j n", j=DT),
                func=AF.Copy,
            )

        def emit_vproj(sb):
            # project the downsampled values: out = aoT @ Wv + qTp (identity-
            # matmul step: the residual add costs one more PE accumulation
            # instead of a DVE op on the PSUM)
            r = resid_p.tile([128, DT, 512], MM_DT, tag="residT", name="residT")
            residT[sb] = r
            for do in range(DT):
                ps = ps_proj.tile([128, 512], FP32, tag="proj_ps", name="vds_ps")
                for ki in range(DT):
                    nc.tensor.matmul(
                        ps, lhsT=wv_sb[:, ki, ts(do, 128)], rhs=aoT[sb][:, ki, :],
                        start=(ki == 0), stop=False,
                    )
                nc.tensor.matmul(
                    ps, lhsT=identity, rhs=qTp[:, do, ts(sb, 512)],
                    start=False, stop=True,
                )
                nc.scalar.activation(out=r[:, do, :], in_=ps, func=AF.Copy)

        def emit_ln1(sb):
            sq = resid_p.tile([128, DT, 512], MM_DT, tag="sq1")
            stats = small.tile([1, 1024], MM_DT, tag="stats1")

            def write_x(dt, src, sb=sb):
                if sb == 0:
                    # affine on DVE (2-scalar tensor_scalar) - ACT is the
                    # hottest engine in this window
                    nc.vector.tensor_scalar(
                        out=xT[:, dt, ts(sb, 512)], in0=src,
                        scalar1=g1_sb[:, dt : dt + 1], scalar2=gb1_sb[:, dt : dt + 1],
                        op0=OP.mult, op1=OP.add,
                    )
                else:
                    nc.scalar.activation(
                        out=xT[:, dt, ts(sb, 512)], in_=src, func=AF.Identity,
                        bias=gb1_sb[:, dt : dt + 1], scale=g1_sb[:, dt : dt + 1],
                    )

            _emit_ln_T(nc, P, residT[sb][:], sq, stats, write_x, pool_only=(sb == 0))

        def emit_ffn(blk, c0=0, n=512):
            col = blk * 512 + c0
            hT = hT_p.tile([128, DT, 512], MM_DT, tag="hT")
            for ht in range(DT):
                ps = ps_proj.tile([128, 512], FP32, tag="proj_ps", name="ffn1_ps")
                for ki in range(DT):
                    nc.tensor.matmul(
                        ps[:, :n], lhsT=late["w1"][:, ki, ts(ht, 128)],
                        rhs=xT[:, ki, ds(col, n)],
                        start=(ki == 0), stop=(ki == DT - 1),
                    )
                nc.scalar.activation(
                    out=hT[:, ht, :n], in_=ps[:, :n], func=AF.Relu,
                    bias=late["b1"][:, ht : ht + 1], scale=1.0,
                )
            resid2 = resid_p.tile([128, DT, 512], MM_DT, tag="resid2")
            for dt in range(DT):
                ps = ps_proj.tile([128, 512], FP32, tag="proj_ps", name="ffn2_ps")
                for hi in range(DT):
                    nc.tensor.matmul(
                        ps[:, :n], lhsT=late["w2"][:, hi, ts(dt, 128)],
                        rhs=hT[:, hi, :n],
                        start=(hi == 0), stop=(hi == DT - 1),
                    )
                nc.vector.scalar_tensor_tensor(
                    out=resid2[:, dt, :n], in0=ps[:, :n], scalar=late["b2"][:, dt : dt + 1],
                    in1=xT[:, dt, ds(col, n)], op0=OP.add, op1=OP.add,
                )
            sq2 = hT_p.tile([128, DT, 512], MM_DT, tag="sq2")
            stats2 = small.tile([1, 1024], MM_DT, tag="stats2")
            out_sb = out_p.tile([128, DT, 512], MM_DT, tag="out_sb")

            def write_out(dt, src, out_sb=out_sb, col=col, n=n):
                nc.scalar.activation(
                    out=out_sb[:, dt, :n], in_=src, func=AF.Identity,
                    bias=late["gb2"][:, dt : dt + 1], scale=late["g2"][:, dt : dt + 1],
                )
                # per-dt chunked writeback overlaps the remaining apply work
                nc.sync.dma_start(out=outT_t[:, dt, ds(col, n)], in_=out_sb[:, dt, :n])

            _emit_ln_T(nc, P, resid2[:, :, :n], sq2[:, :, :n], stats2, write_out, n=n)

        # mids lag TWO blocks: block b-2's stt chain (4us of serial DVE) is
        # guaranteed drained, so the transposes never stall the PE
        accs = {}
        for b in range(NBLK):
            kv = emit_kproj(b)
            if b == 0:
                load_late_consts()
            if b >= 2:
                emit_mid(b - 2, accs.pop(b - 2))
            accs[b] = emit_scores(b, *kv)
            if b == 5:
                emit_vproj(0)
                emit_ln1(0)
        emit_mid(NBLK - 2, accs.pop(NBLK - 2))
        emit_mid(NBLK - 1, accs.pop(NBLK - 1))
        emit_vproj(NSB - 1)
        emit_ln1(NSB - 1)
        # FFN(0) is fully ready here; its matmuls fill the PE while
        # LN1(1)'s DVE/ACT chain drains.
        emit_ffn(0)
        emit_ffn(NSB - 1)

    nc.finalize()
    return nc


def kernel(**inputs):
    has_bv = bool(np.any(np.asarray(inputs["b_v"], dtype=np.float32) != 0.0))
    key = ("prog", has_bv)
    if key not in _CACHE:
        _CACHE[key] = build_program(has_bv)
    nc = _CACHE[key]

    import ml_dtypes

    f32 = lambda x: np.ascontiguousarray(np.asarray(x), dtype=np.float32)
    bf16 = lambda x: np.ascontiguousarray(np.asarray(x, dtype=np.float32).astype(ml_dtypes.bfloat16))
    fp8 = lambda x: np.ascontiguousarray(np.asarray(x, dtype=np.float32).astype(ml_dtypes.float8_e4m3))
    query, key_, value = f32(inputs["query"]), f32(inputs["key"]), f32(inputs["value"])

    shared = {
        n: f32(inputs[n])
        for n in ("b_q", "b_k", "ffn_b1", "ffn_b2",
                  "ln1_g", "ln1_b", "ln2_g", "ln2_b")
    }
    for n in ("w_q", "w_v", "ffn_w1", "ffn_w2"):
        shared[n] = bf16(inputs[n])
    shared["w_k"] = fp8(inputs["w_k"])
    if has_bv:
        wv64 = np.asarray(inputs["w_v"], dtype=np.float64)
        u = np.linalg.solve(wv64.T, np.asarray(inputs["b_v"], dtype=np.float64))
        shared["ufold"] = bf16(u)
    p_idx = np.arange(128)[:, None]
    k_idx = np.arange(512)[None, :]
    shared["cmask"] = ((k_idx - 4 * p_idx >= 0) & (k_idx - 4 * p_idx <= 3)).astype(np.float32)
    shared["cident"] = bf16(np.eye(128))

    in_maps = []
    for c in range(NCORES):
        bi, half = c // 2, c % 2
        w0 = half * WPC
        m = dict(shared)
        m["qT"] = bf16(query[bi, w0 : w0 + WPC, :].T)
        m["kT"] = fp8(key_[bi, w0 * F : (w0 + WPC) * F, :].T)
        m["vP"] = bf16(value[bi, w0 * F : (w0 + WPC) * F, :].reshape(WPC, F * D))
        in_maps.append(m)

    res = run_bass_kernel_spmd(nc, in_maps, core_ids=list(range(NCORES)))
    _CACHE["last_result"] = res
    out = np.empty((B, SQ, D), dtype=np.float32)
    for c in range(NCORES):
        bi, half = c // 2, c % 2
        w0 = half * WPC
        out[bi, w0 : w0 + WPC, :] = res.results[c]["outT"].T.astype(np.float32)
    return out

